# revision 2
# baseline (speedup 1.0000x reference)
"""MoE FeedForward (top-2 of 4 experts) — expert-parallel Trainium2 kernel.

Strategy: the tiny gating matmul + top-k routing run on host as part of input
sharding; tokens are dispatched by gate index to expert-owning cores (expert
e -> cores 2e, 2e+1, each taking half of that expert's tokens, padded to a
common capacity C=2048). Each core computes
    y^T = (relu(W1^T @ x^T) -> W2^T @ mid) (gates applied on host at combine)
entirely in transposed [feature, token] layout with bf16 matmuls accumulating
in fp32 PSUM; the host combine scatter-adds the gate-weighted contributions.

Speed structure (from perfetto analysis; ~124us vs ~109us pure-PE floor):
- fp8 tail pair: the LAST 2 of 16 GEMM2 h-tiles run as ONE fp8e4 DoubleRow
  matmul per d-tile (2 weights/cell, 2 MACs/cycle) — saves ~3us of PE time.
  Scales are baked on host (w1 trailing cols x S_MID, packed w2f8 x S_W2,
  S_MID*S_W2=1) so no on-device scaling ops exist; measured end-to-end
  rel err 1.39e-2 vs the 2e-2 budget (fp8 error ~ full-fp8-err * sqrt(2/16)).
- head schedule: warm-up matmuls start at the post-preamble barrier (~7.1us)
  so the HAM clock-gate un-throttles ~4.7us later; the first DMA pair carries
  w1 h-tiles 0-2 + chunk0 tokens; remaining w1 arrives as three need-ordered
  pieces. Concurrent DMA copies share HBM bandwidth (packet round-robin), so
  from the 3rd load on, each load k waits for load k-2 via a tiny vector-
  engine "corner write" into its destination (WAW) — capping in-flight
  copies at ~2 in need order. Plain read-gates do NOT work: the scheduler
  hoists independent DMAs past them.
- tail: the final di's PSUM drain splits across scalar ACT + vector ADD with
  the two output DMAs on the scalar and sync rings in parallel.

Model dims (hardcoded per problem spec): N=8192 tokens, D=512, H=2048,
E=4 experts, top-k=2, 8 NeuronCores.
"""

import numpy as np
import ml_dtypes
from contextlib import ExitStack

D = 512
H = 2048
E = 4
TOP_K = 2
N_CORES = 8
ND = D // 128   # 4 d-tiles
NH = H // 128   # 16 h-tiles
S0 = 320        # first-chunk width
N_WARM = 68     # PE warm-up matmuls bridging preamble -> head DMA landing
CAP = 2048      # per-core token capacity; overflow handled on host
N_F8_HT = 2     # trailing GEMM2 h-tiles computed in fp8 DoubleRow (must be even)
F8_HT0 = NH - N_F8_HT
S_MID = 0.25    # mid-activation scale for fp8 h-tiles (baked into w1 on host)
S_W2 = 4.0      # w2 scale for fp8 h-tiles (baked into packed w2f8; S_MID*S_W2=1)
WARM_MEMSET = True   # seed warm tile (sim rejects uninitialized reads)

# w1 ladder pieces (h-tile ranges) after the head's ht0-1
W1_PIECES = [(3, 6), (7, 11), (12, 15)]
N_HEAD_HT = 3   # h-tiles carried in head_a
N_HEAD_DI = 2   # xt0 di-blocks carried in head_a (rest in head_b)

_NC_CACHE = {}


def _chunk_plan(C: int):
    if C <= S0 + 224:
        return [(0, C)]
    chunks = [(0, S0)]
    off = S0
    while C - off > 704 + 256:
        chunks.append((off, 512))
        off += 512
    rem = C - off
    if rem > 512:
        a = (rem - 256) // 2 * 2
        chunks.append((off, a))
        chunks.append((off + a, rem - a))
    else:
        chunks.append((off, rem))
    return chunks


def _build_moe_nc(C: int):
    """Per-core SPMD program: [D,C] bf16 tokens -> [D,C] bf16 expert output."""
    import concourse.mybir as mybir
    from concourse import bacc, tile

    dt = mybir.dt
    AF = mybir.ActivationFunctionType

    assert C % 2 == 0
    chunks = _chunk_plan(C)
    s0 = chunks[0][1]

    nc = bacc.Bacc(None, target_bir_lowering=False)
    # host pre-arranges every input partition-major so each DMA below is a
    # flat, fully contiguous [128, K] copy.
    # head_a: [w1 ht0-1 (ht-major, di inside) | xt0 di0]; head_b: [xt0 di1..3]
    head_a = nc.dram_tensor("head_a",
                            [128, N_HEAD_HT * ND * 128 + N_HEAD_DI * s0],
                            dt.bfloat16, kind="ExternalInput")
    head_b = nc.dram_tensor("head_b", [128, (ND - N_HEAD_DI) * s0],
                            dt.bfloat16, kind="ExternalInput")
    # w1 ladder: one dram tensor per piece, di-major inside each h-tile
    w1p = {}
    for (a, b) in W1_PIECES:
        w1p[(a, b)] = nc.dram_tensor(f"w1p_{a}", [128, (b - a + 1) * ND * 128],
                                     dt.bfloat16, kind="ExternalInput")
    if C > s0:
        xt = nc.dram_tensor("xt", [128, ND * (C - s0)], dt.bfloat16,
                            kind="ExternalInput")
    w2 = nc.dram_tensor("w2", [128, (8 + 8 - N_F8_HT) * 512], dt.bfloat16,
                        kind="ExternalInput")
    w2f8 = nc.dram_tensor("w2f8", [128, N_F8_HT, 512], dt.float8e4,
                          kind="ExternalInput")
    # output, chunk-major: per chunk a [128, ND*S] contiguous block
    yt = nc.dram_tensor("yt", [128, ND * C], dt.bfloat16, kind="ExternalOutput")

    # flat offset of each chunk's block inside xt (chunk0 lives in head) / yt
    xt_off = {}
    yt_off = {}
    acc = 0
    yacc = 0
    for i, (c0, S) in enumerate(chunks):
        if i > 0:
            xt_off[c0] = acc
            acc += ND * S
        yt_off[c0] = yacc
        yacc += ND * S

    with tile.TileContext(nc) as tc, ExitStack() as ctx:
        wpool = ctx.enter_context(tc.tile_pool(name="weights", bufs=1))
        xpool = ctx.enter_context(tc.tile_pool(name="x", bufs=1))
        midp = ctx.enter_context(tc.tile_pool(name="mid", bufs=34))
        p1 = ctx.enter_context(tc.tile_pool(name="p1", bufs=5, space="PSUM"))
        p2 = ctx.enter_context(tc.tile_pool(name="p2", bufs=2, space="PSUM"))
        pw = ctx.enter_context(tc.tile_pool(name="pw", bufs=1, space="PSUM"))
        ypool = ctx.enter_context(tc.tile_pool(name="y", bufs=6))

        # PE warm-up: dummy matmuls spanning the first input DMA's flight
        # time so the HAM clock gate un-throttles as early as possible.
        # The tile is read uninitialized on purpose: the output bank is
        # never read, so the values are irrelevant.
        warm_sb = wpool.tile([128, 128], dt.bfloat16, tag="warm", name="warm_sb")
        if WARM_MEMSET:
            nc.gpsimd.memset(warm_sb[:], 0.0)
        warm_ps = pw.tile([128, 64], dt.float32, tag="warm_ps", name="warm_ps")
        for _ in range(N_WARM):
            nc.tensor.matmul(warm_ps[:], warm_sb[:], warm_sb[:, :64],
                             start=True, stop=True, skip_group_check=True)

        # ---- input loads: need-ordered ladder on the sync HWDGE ring ----
        # Concurrent DMA copies share HBM bandwidth ~equally (packet-level
        # round-robin across queues), so an uncontrolled pile-up starves the
        # earliest (most urgent) pieces. The head pair + first two w1 pieces
        # flow freely (they complete in sequence); from the third load on,
        # each load k's destination gets a tiny SBUF->SBUF "corner write"
        # from load k-2's tile before the real dma_start: the WAW hazard on
        # the corner makes load k wait until load k-2 has fully landed,
        # capping in-flight transfers at ~2. (Plain read-gates don't work:
        # the scheduler hoists independent DMAs past a blocked gate.)
        chain = []   # landed-order history of chained load tiles

        def corner(t):
            return t[:, 0, :2] if len(t.shape) == 3 else t[:, :2]

        def chained_load(t, dram_ap, depth=2):
            if len(chain) >= depth:
                # cheap corner write on the (idle) vector engine -- a DMA
                # corner write would add its own ~1.7us completion latency
                src = chain[-depth]
                nc.vector.tensor_scalar_add(corner(t), corner(src), 0.0)
            nc.sync.dma_start(t[:], dram_ap)
            chain.append(t)

        head_a_sb = wpool.tile([128, N_HEAD_HT * ND * 128 + N_HEAD_DI * s0],
                               dt.bfloat16, tag="head_a", name="head_a_sb")
        nc.sync.dma_start(head_a_sb[:], head_a[:])
        head_b_sb = wpool.tile([128, (ND - N_HEAD_DI) * s0], dt.bfloat16,
                               tag="head_b", name="head_b_sb")
        nc.sync.dma_start(head_b_sb[:], head_b[:])

        w1_tile = {}   # ht -> (sbuf tile, col base)

        for (a, b) in W1_PIECES:
            t = wpool.tile([128, (b - a + 1) * ND * 128], dt.bfloat16,
                           tag=f"w1p_{a}", name=f"w1p_{a}")
            chained_load(t, w1p[(a, b)][:])
            for j in range(a, b + 1):
                w1_tile[j] = (t, (j - a) * ND * 128)

        xt_sb = {}
        xt_sb[chunks[0][0]] = None  # chunk0 rides in head_a/head_b

        def load_xt(cs):
            # one DMA covering chunks cs (contiguous in xt layout)
            w = sum(ND * S for (_, S) in cs)
            t = xpool.tile([128, w], dt.bfloat16,
                           tag=f"xt_{cs[0][0]}", name=f"xt_{cs[0][0]}")
            o = xt_off[cs[0][0]]
            chained_load(t, xt[:, o:o + w])
            base = 0
            for (c0, S) in cs:
                xt_sb[c0] = (t, base)
                base += ND * S

        w2_sb = []

        def load_w2(wb):
            n_ht = 8 if wb == 0 else 8 - N_F8_HT
            t = wpool.tile([128, n_ht * 512], dt.bfloat16,
                           tag=f"w2_{wb}", name=f"w2_{wb}")
            o = wb * 8 * 512
            chained_load(t, w2[:, o:o + n_ht * 512])
            w2_sb.append(t)

        rest = chunks[1:]
        if len(rest) >= 1:
            load_xt(rest[:1])
        load_w2(0)
        if len(rest) >= 2:
            load_xt(rest[1:2])
        load_w2(1)
        if len(rest) >= 3:
            load_xt(rest[2:])
        w2f8_sb = wpool.tile([128, N_F8_HT, 512], dt.float8e4,
                             tag="w2f8", name="w2f8_sb")
        chained_load(w2f8_sb, w2f8[:])

        def w1_lhsT(ht, di):
            if ht < N_HEAD_HT:
                o = (ht * ND + di) * 128
                return head_a_sb[:, o:o + 128]
            t, base = w1_tile[ht]
            o = base + di * 128
            return t[:, o:o + 128]

        def gemm1(c0, S):
            # mid^T[h, c] = relu(sum_d w1[d,h] * x^T[d,c]); trailing
            # N_F8_HT h-tiles emit fp8e4 mids (pre-scaled by S_MID via w1)
            # into one pair-blocked [128, N_F8_HT, 512] tile for DoubleRow.
            mids = []
            m8 = midp.tile([128, N_F8_HT, 512], dt.float8e4, tag="mid8",
                           name=f"mid8_{c0}")
            for ht in range(NH):
                ps = p1.tile([128, 512], dt.float32, tag="ps1",
                             name=f"ps1_{c0}_{ht}")
                for di in range(ND):
                    if xt_sb[c0] is None:   # chunk0 rides in the head tiles
                        if di < N_HEAD_DI:
                            ho = N_HEAD_HT * ND * 128
                            rhs = head_a_sb[:, ho + di * S:ho + (di + 1) * S]
                        else:
                            dj = di - N_HEAD_DI
                            rhs = head_b_sb[:, dj * S:(dj + 1) * S]
                    else:
                        t, base = xt_sb[c0]
                        rhs = t[:, base + di * S:base + (di + 1) * S]
                    nc.tensor.matmul(
                        ps[:, :S],
                        w1_lhsT(ht, di),
                        rhs,
                        start=(di == 0),
                        stop=(di == ND - 1),
                    )
                if ht >= F8_HT0:
                    nc.scalar.activation(m8[:, ht - F8_HT0, :S], ps[:, :S],
                                         AF.Relu)
                else:
                    m = midp.tile([128, 512], dt.bfloat16, tag="mid",
                                  name=f"mid_{c0}_{ht}")
                    nc.scalar.activation(m[:, :S], ps[:, :S], AF.Relu)
                    mids.append(m)
            return (mids, m8)

        def gemm2(c0, S, mids8, last=False):
            # y^T[d, c] = sum_h w2[h,d] * mid^T[h,c]
            mids, m8 = mids8
            o = yt_off[c0]
            for di in range(ND):
                ps2 = p2.tile([128, 512], dt.float32, tag="ps2",
                              name=f"ps2_{c0}_{di}")
                for ht in range(F8_HT0):
                    wo = (ht % 8) * 512 + di * 128
                    nc.tensor.matmul(
                        ps2[:, :S],
                        w2_sb[ht // 8][:, wo:wo + 128],
                        mids[ht][:, :S],
                        start=(ht == 0),
                        stop=False,
                    )
                # trailing h-tile pairs: one fp8 DoubleRow matmul per pair
                # contracts two k-tiles (2 fp8 weights/cell, 2 MACs/cycle)
                for pi in range(N_F8_HT // 2):
                    nc.tensor.matmul(
                        ps2[:, :S],
                        w2f8_sb[:, 2 * pi:2 * pi + 2, di * 128:(di + 1) * 128],
                        m8[:, 2 * pi:2 * pi + 2, :S],
                        start=False,
                        stop=(pi == N_F8_HT // 2 - 1),
                        perf_mode=mybir.MatmulPerfMode.DoubleRow,
                    )
                yt_t = ypool.tile([128, 512], dt.bfloat16, tag="y",
                                  name=f"y_{c0}_{di}")
                if last and di == ND - 1:
                    # kernel tail: drain the two halves in parallel -- scalar
                    # ACT + scalar-ring DMA for half 0, vector ADD + sync-ring
                    # DMA for half 1 -- so the last DMA issues ~0.4us sooner.
                    hf = S // 2
                    nc.scalar.activation(yt_t[:, :hf], ps2[:, :hf], AF.Copy)
                    nc.scalar.dma_start(yt[:, o + di * S:o + di * S + hf],
                                        yt_t[:, :hf])
                    nc.vector.tensor_scalar_add(yt_t[:, hf:S], ps2[:, hf:S],
                                                0.0)
                    nc.sync.dma_start(yt[:, o + di * S + hf:o + (di + 1) * S],
                                      yt_t[:, hf:S])
                else:
                    nc.scalar.activation(yt_t[:, :S], ps2[:, :S], AF.Copy)
                    nc.scalar.dma_start(yt[:, o + di * S:o + (di + 1) * S],
                                        yt_t[:, :S])

        # software-pipeline by one chunk: GEMM1 of chunk i+1 is emitted before
        # GEMM2 of chunk i
        prev = None
        for (c0, S) in chunks:
            mids = gemm1(c0, S)
            if prev is not None:
                gemm2(*prev)
            prev = (c0, S, mids)
        gemm2(*prev, last=True)

    nc.finalize()
    return nc


def _route(h, w_gate):
    """Top-2 gating, matching jax.lax.top_k (ties -> lower index) + softmax."""
    logits = h @ w_gate                                      # [N, E] f32
    order = np.argsort(-logits, axis=1, kind="stable")
    top_idx = order[:, :TOP_K]                               # [N, 2]
    top_lg = np.take_along_axis(logits, top_idx, axis=1)
    mx = top_lg.max(axis=1, keepdims=True)
    ex = np.exp(top_lg - mx)
    gates2 = (ex / ex.sum(axis=1, keepdims=True)).astype(np.float32)
    return top_idx, gates2


def _run(inputs, trace=False):
    from concourse.bass_utils import run_bass_kernel_spmd

    bf16 = ml_dtypes.bfloat16
    h = np.asarray(inputs["h"], dtype=np.float32)
    w_gate = np.asarray(inputs["w_gate"], dtype=np.float32)
    w1 = np.asarray(inputs["w1"], dtype=np.float32)
    b1 = np.asarray(inputs["b1"], dtype=np.float32)
    w2 = np.asarray(inputs["w2"], dtype=np.float32)
    b2 = np.asarray(inputs["b2"], dtype=np.float32)
    N = h.shape[0]

    if b1.any() or b2.any():
        # safety fallback (setup_inputs uses zero biases): exact host compute
        logits = h @ w_gate
        order = np.argsort(-logits, axis=1, kind="stable")
        ti = order[:, :TOP_K]
        tl = np.take_along_axis(logits, ti, axis=1)
        exl = np.exp(tl - tl.max(axis=1, keepdims=True))
        g2 = exl / exl.sum(axis=1, keepdims=True)
        out = np.zeros((N, D), dtype=np.float32)
        for e in range(E):
            sel = ti == e
            toks = np.nonzero(sel.any(axis=1))[0]
            g = g2[toks, sel[toks].argmax(axis=1)]
            mid = np.maximum(h[toks] @ w1[e] + b1[e], 0.0)
            out[toks] += (mid @ w2[e] + b2[e]) * g[:, None]
        return out, None

    top_idx, gates2 = _route(h, w_gate)

    # dispatch: expert e -> cores 2e (first half) and 2e+1 (second half);
    # capacity-limited with host-side overflow handling
    core_toks, core_gates, core_expert, host_jobs = [], [], [], []
    for e in range(E):
        sel = top_idx == e                                   # [N, 2] bool
        toks = np.nonzero(sel.any(axis=1))[0]
        g = gates2[toks, sel[toks].argmax(axis=1)]
        if len(toks) > 2 * CAP:
            host_jobs.append((e, toks[2 * CAP:], g[2 * CAP:]))
            toks, g = toks[:2 * CAP], g[:2 * CAP]
        half = (len(toks) + 1) // 2
        for lo, hi in ((0, half), (half, len(toks))):
            core_toks.append(toks[lo:hi])
            core_gates.append(g[lo:hi])
            core_expert.append(e)

    maxlen = max(len(t) for t in core_toks)
    C = max(128, -(-maxlen // 2) * 2)

    if C not in _NC_CACHE:
        _NC_CACHE[C] = _build_moe_nc(C)
    nc = _NC_CACHE[C]

    chunks = _chunk_plan(C)
    s0 = chunks[0][1]

    # partition-major packers matching the kernel's flat DMA layouts
    def pack_w1(e, h0, h1):
        # di-major inside each h-tile: [128, (h1-h0)/128 * ND * 128]
        w1e = w1[e]
        if h1 > F8_HT0 * 128:
            w1e = w1e.copy()
            w1e[:, F8_HT0 * 128:] *= S_MID
        blk = (w1e.astype(bf16).reshape(ND, 128, NH, 128)
               [:, :, h0 // 128:h1 // 128, :])          # [ND,128,nht,128]
        return np.ascontiguousarray(
            blk.transpose(1, 2, 0, 3).reshape(128, -1))  # ht-major, di inside

    f8e4 = ml_dtypes.float8_e4m3fn
    w1_ht0 = {}
    w1_pieces = {}
    w2_packed = {}
    w2f8_packed = {}
    for e in set(core_expert):
        w1_ht0[e] = pack_w1(e, 0, N_HEAD_HT * 128)
        w1_pieces[e] = {
            (a, b): pack_w1(e, a * 128, (b + 1) * 128) for (a, b) in W1_PIECES}
        wt = w2[e].astype(bf16).reshape(16, 128, 512)   # [ht, k, d]
        w2_packed[e] = np.ascontiguousarray(
            wt[:F8_HT0].transpose(1, 0, 2).reshape(128, F8_HT0 * 512))
        w2f8_packed[e] = np.ascontiguousarray(
            np.clip(w2[e][F8_HT0 * 128:].reshape(N_F8_HT, 128, 512) * S_W2,
                    -240, 240).transpose(1, 0, 2).astype(f8e4))

    in_maps = []
    for c in range(N_CORES):
        e = core_expert[c]
        toks = core_toks[c]
        n = len(toks)
        xtT = np.zeros((D, C), dtype=bf16)
        xtT[:, :n] = h[toks].T.astype(bf16)
        r = xtT.reshape(ND, 128, C)

        def xt_block(c0, S):
            return r[:, :, c0:c0 + S].transpose(1, 0, 2).reshape(128, ND * S)

        hw = N_HEAD_HT * ND * 128
        xt0 = xt_block(*chunks[0]).reshape(128, ND, s0)
        head_a_arr = np.empty((128, hw + N_HEAD_DI * s0), dtype=bf16)
        head_a_arr[:, :hw] = w1_ht0[e]
        head_a_arr[:, hw:] = xt0[:, :N_HEAD_DI, :].reshape(128, -1)
        head_b_arr = np.ascontiguousarray(
            xt0[:, N_HEAD_DI:, :].reshape(128, -1))
        im = {"head_a": head_a_arr, "head_b": head_b_arr,
              "w2": w2_packed[e], "w2f8": w2f8_packed[e]}
        for (a, b) in W1_PIECES:
            im[f"w1p_{a}"] = w1_pieces[e][(a, b)]
        if C > s0:
            xt_arr = np.empty((128, ND * (C - s0)), dtype=bf16)
            o = 0
            for (c0, S) in chunks[1:]:
                xt_arr[:, o:o + ND * S] = xt_block(c0, S)
                o += ND * S
            im["xt"] = xt_arr
        in_maps.append(im)

    res = run_bass_kernel_spmd(nc, in_maps, core_ids=list(range(N_CORES)),
                               trace=trace)

    out = np.zeros((N, D), dtype=np.float32)
    # routing-overflow stragglers: same FFN on host, exact f32
    for e, toks, g in host_jobs:
        mid = np.maximum(h[toks] @ w1[e] + b1[e], 0.0)
        out[toks] += (mid @ w2[e] + b2[e]) * g[:, None]
    for c in range(N_CORES):
        toks = core_toks[c]
        if not len(toks):
            continue
        # unpack chunk-major [128, ND*C] back to y^T [D, C]
        raw = res.results[c]["yt"]
        ytT = np.empty((D, C), dtype=np.float32)
        o = 0
        for (c0, S) in chunks:
            ytT[:, c0:c0 + S] = (
                raw[:, o:o + ND * S].astype(np.float32).reshape(128, ND, S)
                .transpose(1, 0, 2).reshape(D, S))
            o += ND * S
        out[toks] += ytT[:, :len(toks)].T * core_gates[c][:, None]
    return out, res


def kernel(**inputs) -> np.ndarray:
    out, _ = _run(inputs, trace=False)
    return out


# revision 3
# speedup vs baseline: 1.1734x; 1.1734x over previous
"""MoE FeedForward (top-2 of 4 experts) — expert-parallel Trainium2 kernel.

Strategy: the tiny gating matmul + top-k routing run on host as part of input
sharding; tokens are dispatched by gate index to expert-owning cores (expert
e -> cores 2e, 2e+1, each taking half of that expert's tokens, padded to a
common capacity C=2048). Each core computes
    y^T = (relu(W1^T @ x^T) -> W2^T @ mid) (gates applied on host at combine)
entirely in transposed [feature, token] layout with bf16 matmuls accumulating
in fp32 PSUM; the host combine scatter-adds the gate-weighted contributions.

Speed structure (from perfetto analysis; ~124us vs ~109us pure-PE floor):
- fp8 tail pair: the LAST 2 of 16 GEMM2 h-tiles run as ONE fp8e4 DoubleRow
  matmul per d-tile (2 weights/cell, 2 MACs/cycle) — saves ~3us of PE time.
  Scales are baked on host (w1 trailing cols x S_MID, packed w2f8 x S_W2,
  S_MID*S_W2=1) so no on-device scaling ops exist; measured end-to-end
  rel err 1.802e-2 vs the 2e-2 budget (fp8 error ~ full-fp8-err *
  sqrt(ht_fraction * token_fraction)). A second fp8 pair (ht12-13) runs only
  on chunks with S>=448 where DoubleRow actually gains (at small S the
  256-col LDWEIGHTS is exposed); small chunks keep ht12-13 in bf16 -- their
  w1 cols are pre-scaled 1/4 and the bf16 w2 rows carry the inverse 4x,
  both bf16-lossless powers of 2, so the two paths share packed weights.
- head schedule: warm-up matmuls start at the post-preamble barrier (~7.1us)
  so the HAM clock-gate un-throttles ~4.7us later; the first DMA pair carries
  w1 h-tiles 0-2 + chunk0 tokens; remaining w1 arrives as three need-ordered
  pieces. Concurrent DMA copies share HBM bandwidth (packet round-robin), so
  from the 3rd load on, each load k waits for load k-2 via a tiny vector-
  engine "corner write" into its destination (WAW) — capping in-flight
  copies at ~2 in need order. Plain read-gates do NOT work: the scheduler
  hoists independent DMAs past them.
- tail: the final di's PSUM drain splits across scalar ACT + vector ADD with
  the two output DMAs on the scalar and sync rings in parallel.

Model dims (hardcoded per problem spec): N=8192 tokens, D=512, H=2048,
E=4 experts, top-k=2, 8 NeuronCores.
"""

import numpy as np
import ml_dtypes
from contextlib import ExitStack

D = 512
H = 2048
E = 4
TOP_K = 2
N_CORES = 8
ND = D // 128   # 4 d-tiles
NH = H // 128   # 16 h-tiles
S0 = 320        # first-chunk width
N_WARM = 68     # PE warm-up matmuls bridging preamble -> head DMA landing
CAP = 2048      # per-core token capacity; overflow handled on host
N_F8_HT = 4     # trailing GEMM2 h-tiles eligible for fp8 DoubleRow (even)
F8_HT0 = NH - N_F8_HT
F8_BIG = 448    # chunks with S >= this run 2 fp8 pairs; smaller run 1 pair
                # (pair ht14-15 always fp8; pair ht12-13 fp8 on big chunks only,
                # bf16 elsewhere -- DR gains ~nothing at small S due to exposed
                # LDWEIGHTS, so this buys error margin for free)
S_MID = 0.25    # mid scale for ht12-15 (baked into w1 on host); w2 bf16 rows
                # ht12-13 carry the inverse 4x -- powers of 2 are bf16-lossless
WARM_MEMSET = True   # seed warm tile (sim rejects uninitialized reads)

# w1 ladder pieces (h-tile ranges) after the head's ht0-1
W1_PIECES = [(3, 6), (7, 11), (12, 15)]
N_HEAD_HT = 3   # h-tiles carried in head_a
N_HEAD_DI = 2   # xt0 di-blocks carried in head_a (rest in head_b)

_NC_CACHE = {}


def _chunk_plan(C: int):
    if C <= S0 + 224:
        return [(0, C)]
    chunks = [(0, S0)]
    off = S0
    while C - off > 704 + 256:
        chunks.append((off, 512))
        off += 512
    rem = C - off
    if rem > 512:
        a = (rem - 256) // 2 * 2
        chunks.append((off, a))
        chunks.append((off + a, rem - a))
    else:
        chunks.append((off, rem))
    return chunks


def _build_moe_nc(C: int):
    """Per-core SPMD program: [D,C] bf16 tokens -> [D,C] bf16 expert output."""
    import concourse.mybir as mybir
    from concourse import bacc, tile

    dt = mybir.dt
    AF = mybir.ActivationFunctionType

    assert C % 2 == 0
    chunks = _chunk_plan(C)
    s0 = chunks[0][1]

    nc = bacc.Bacc(None, target_bir_lowering=False)
    # host pre-arranges every input partition-major so each DMA below is a
    # flat, fully contiguous [128, K] copy.
    # head_a: [w1 ht0-1 (ht-major, di inside) | xt0 di0]; head_b: [xt0 di1..3]
    head_a = nc.dram_tensor("head_a",
                            [128, N_HEAD_HT * ND * 128 + N_HEAD_DI * s0],
                            dt.bfloat16, kind="ExternalInput")
    head_b = nc.dram_tensor("head_b", [128, (ND - N_HEAD_DI) * s0],
                            dt.bfloat16, kind="ExternalInput")
    # w1 ladder: one dram tensor per piece, di-major inside each h-tile
    w1p = {}
    for (a, b) in W1_PIECES:
        w1p[(a, b)] = nc.dram_tensor(f"w1p_{a}", [128, (b - a + 1) * ND * 128],
                                     dt.bfloat16, kind="ExternalInput")
    if C > s0:
        xt = nc.dram_tensor("xt", [128, ND * (C - s0)], dt.bfloat16,
                            kind="ExternalInput")
    w2 = nc.dram_tensor("w2", [128, 14 * 512], dt.bfloat16,
                        kind="ExternalInput")
    w2f8 = nc.dram_tensor("w2f8", [128, N_F8_HT, 512], dt.float8e4,
                          kind="ExternalInput")
    # output, chunk-major: per chunk a [128, ND*S] contiguous block
    yt = nc.dram_tensor("yt", [128, ND * C], dt.bfloat16, kind="ExternalOutput")

    # flat offset of each chunk's block inside xt (chunk0 lives in head) / yt
    xt_off = {}
    yt_off = {}
    acc = 0
    yacc = 0
    for i, (c0, S) in enumerate(chunks):
        if i > 0:
            xt_off[c0] = acc
            acc += ND * S
        yt_off[c0] = yacc
        yacc += ND * S

    with tile.TileContext(nc) as tc, ExitStack() as ctx:
        wpool = ctx.enter_context(tc.tile_pool(name="weights", bufs=1))
        xpool = ctx.enter_context(tc.tile_pool(name="x", bufs=1))
        midp = ctx.enter_context(tc.tile_pool(name="mid", bufs=34))
        p1 = ctx.enter_context(tc.tile_pool(name="p1", bufs=5, space="PSUM"))
        p2 = ctx.enter_context(tc.tile_pool(name="p2", bufs=2, space="PSUM"))
        pw = ctx.enter_context(tc.tile_pool(name="pw", bufs=1, space="PSUM"))
        ypool = ctx.enter_context(tc.tile_pool(name="y", bufs=6))

        # PE warm-up: dummy matmuls spanning the first input DMA's flight
        # time so the HAM clock gate un-throttles as early as possible.
        # The tile is read uninitialized on purpose: the output bank is
        # never read, so the values are irrelevant.
        warm_sb = wpool.tile([128, 128], dt.bfloat16, tag="warm", name="warm_sb")
        if WARM_MEMSET:
            nc.gpsimd.memset(warm_sb[:], 0.0)
        warm_ps = pw.tile([128, 64], dt.float32, tag="warm_ps", name="warm_ps")
        for _ in range(N_WARM):
            nc.tensor.matmul(warm_ps[:], warm_sb[:], warm_sb[:, :64],
                             start=True, stop=True, skip_group_check=True)

        # ---- input loads: need-ordered ladder on the sync HWDGE ring ----
        # Concurrent DMA copies share HBM bandwidth ~equally (packet-level
        # round-robin across queues), so an uncontrolled pile-up starves the
        # earliest (most urgent) pieces. The head pair + first two w1 pieces
        # flow freely (they complete in sequence); from the third load on,
        # each load k's destination gets a tiny SBUF->SBUF "corner write"
        # from load k-2's tile before the real dma_start: the WAW hazard on
        # the corner makes load k wait until load k-2 has fully landed,
        # capping in-flight transfers at ~2. (Plain read-gates don't work:
        # the scheduler hoists independent DMAs past a blocked gate.)
        chain = []   # landed-order history of chained load tiles

        def corner(t):
            return t[:, 0, :2] if len(t.shape) == 3 else t[:, :2]

        def chained_load(t, dram_ap, depth=2):
            if len(chain) >= depth:
                # cheap corner write on the (idle) vector engine -- a DMA
                # corner write would add its own ~1.7us completion latency
                src = chain[-depth]
                nc.vector.tensor_scalar_add(corner(t), corner(src), 0.0)
            nc.sync.dma_start(t[:], dram_ap)
            chain.append(t)

        head_a_sb = wpool.tile([128, N_HEAD_HT * ND * 128 + N_HEAD_DI * s0],
                               dt.bfloat16, tag="head_a", name="head_a_sb")
        nc.sync.dma_start(head_a_sb[:], head_a[:])
        head_b_sb = wpool.tile([128, (ND - N_HEAD_DI) * s0], dt.bfloat16,
                               tag="head_b", name="head_b_sb")
        nc.sync.dma_start(head_b_sb[:], head_b[:])

        w1_tile = {}   # ht -> (sbuf tile, col base)

        for (a, b) in W1_PIECES:
            t = wpool.tile([128, (b - a + 1) * ND * 128], dt.bfloat16,
                           tag=f"w1p_{a}", name=f"w1p_{a}")
            chained_load(t, w1p[(a, b)][:])
            for j in range(a, b + 1):
                w1_tile[j] = (t, (j - a) * ND * 128)

        xt_sb = {}
        xt_sb[chunks[0][0]] = None  # chunk0 rides in head_a/head_b

        def load_xt(cs):
            # one DMA covering chunks cs (contiguous in xt layout)
            w = sum(ND * S for (_, S) in cs)
            t = xpool.tile([128, w], dt.bfloat16,
                           tag=f"xt_{cs[0][0]}", name=f"xt_{cs[0][0]}")
            o = xt_off[cs[0][0]]
            chained_load(t, xt[:, o:o + w])
            base = 0
            for (c0, S) in cs:
                xt_sb[c0] = (t, base)
                base += ND * S

        w2_sb = []

        def load_w2(wb):
            n_ht = 8 if wb == 0 else 6
            t = wpool.tile([128, n_ht * 512], dt.bfloat16,
                           tag=f"w2_{wb}", name=f"w2_{wb}")
            o = wb * 8 * 512
            chained_load(t, w2[:, o:o + n_ht * 512])
            w2_sb.append(t)

        rest = chunks[1:]
        if len(rest) >= 1:
            load_xt(rest[:1])
        load_w2(0)
        if len(rest) >= 2:
            load_xt(rest[1:2])
        load_w2(1)
        if len(rest) >= 3:
            load_xt(rest[2:])
        w2f8_sb = wpool.tile([128, N_F8_HT, 512], dt.float8e4,
                             tag="w2f8", name="w2f8_sb")
        chained_load(w2f8_sb, w2f8[:])

        def w1_lhsT(ht, di):
            if ht < N_HEAD_HT:
                o = (ht * ND + di) * 128
                return head_a_sb[:, o:o + 128]
            t, base = w1_tile[ht]
            o = base + di * 128
            return t[:, o:o + 128]

        def gemm1(c0, S):
            # mid^T[h, c] = relu(sum_d w1[d,h] * x^T[d,c]); trailing
            # N_F8_HT h-tiles emit fp8e4 mids (pre-scaled by S_MID via w1)
            # into one pair-blocked [128, N_F8_HT, 512] tile for DoubleRow.
            mids = []
            f8start = F8_HT0 if S >= F8_BIG else NH - 2
            m8 = midp.tile([128, N_F8_HT, 512], dt.float8e4, tag="mid8",
                           name=f"mid8_{c0}")
            for ht in range(NH):
                ps = p1.tile([128, 512], dt.float32, tag="ps1",
                             name=f"ps1_{c0}_{ht}")
                for di in range(ND):
                    if xt_sb[c0] is None:   # chunk0 rides in the head tiles
                        if di < N_HEAD_DI:
                            ho = N_HEAD_HT * ND * 128
                            rhs = head_a_sb[:, ho + di * S:ho + (di + 1) * S]
                        else:
                            dj = di - N_HEAD_DI
                            rhs = head_b_sb[:, dj * S:(dj + 1) * S]
                    else:
                        t, base = xt_sb[c0]
                        rhs = t[:, base + di * S:base + (di + 1) * S]
                    nc.tensor.matmul(
                        ps[:, :S],
                        w1_lhsT(ht, di),
                        rhs,
                        start=(di == 0),
                        stop=(di == ND - 1),
                    )
                if ht >= f8start:
                    nc.scalar.activation(m8[:, ht - F8_HT0, :S], ps[:, :S],
                                         AF.Relu)
                else:
                    m = midp.tile([128, 512], dt.bfloat16, tag="mid",
                                  name=f"mid_{c0}_{ht}")
                    nc.scalar.activation(m[:, :S], ps[:, :S], AF.Relu)
                    mids.append(m)
            return (mids, m8)

        def gemm2(c0, S, mids8, last=False):
            # y^T[d, c] = sum_h w2[h,d] * mid^T[h,c]
            mids, m8 = mids8
            f8start = F8_HT0 if S >= F8_BIG else NH - 2
            o = yt_off[c0]
            for di in range(ND):
                ps2 = p2.tile([128, 512], dt.float32, tag="ps2",
                              name=f"ps2_{c0}_{di}")
                for ht in range(f8start):
                    wo = (ht % 8) * 512 + di * 128
                    nc.tensor.matmul(
                        ps2[:, :S],
                        w2_sb[ht // 8][:, wo:wo + 128],
                        mids[ht][:, :S],
                        start=(ht == 0),
                        stop=False,
                    )
                # trailing h-tile pairs: one fp8 DoubleRow matmul per pair
                # contracts two k-tiles (2 fp8 weights/cell, 2 MACs/cycle)
                for pi in range((f8start - F8_HT0) // 2, N_F8_HT // 2):
                    nc.tensor.matmul(
                        ps2[:, :S],
                        w2f8_sb[:, 2 * pi:2 * pi + 2, di * 128:(di + 1) * 128],
                        m8[:, 2 * pi:2 * pi + 2, :S],
                        start=False,
                        stop=(pi == N_F8_HT // 2 - 1),
                        perf_mode=mybir.MatmulPerfMode.DoubleRow,
                    )
                yt_t = ypool.tile([128, 512], dt.bfloat16, tag="y",
                                  name=f"y_{c0}_{di}")
                if last and di == ND - 1:
                    # kernel tail: drain the two halves in parallel -- scalar
                    # ACT + scalar-ring DMA for half 0, vector ADD + sync-ring
                    # DMA for half 1 -- so the last DMA issues ~0.4us sooner.
                    hf = S // 2
                    nc.scalar.activation(yt_t[:, :hf], ps2[:, :hf], AF.Copy)
                    nc.scalar.dma_start(yt[:, o + di * S:o + di * S + hf],
                                        yt_t[:, :hf])
                    nc.vector.tensor_scalar_add(yt_t[:, hf:S], ps2[:, hf:S],
                                                0.0)
                    nc.sync.dma_start(yt[:, o + di * S + hf:o + (di + 1) * S],
                                      yt_t[:, hf:S])
                else:
                    nc.scalar.activation(yt_t[:, :S], ps2[:, :S], AF.Copy)
                    nc.scalar.dma_start(yt[:, o + di * S:o + (di + 1) * S],
                                        yt_t[:, :S])

        # software-pipeline by one chunk: GEMM1 of chunk i+1 is emitted before
        # GEMM2 of chunk i
        prev = None
        for (c0, S) in chunks:
            mids = gemm1(c0, S)
            if prev is not None:
                gemm2(*prev)
            prev = (c0, S, mids)
        gemm2(*prev, last=True)

    nc.finalize()
    return nc


def _route(h, w_gate):
    """Top-2 gating, matching jax.lax.top_k (ties -> lower index) + softmax."""
    logits = h @ w_gate                                      # [N, E] f32
    order = np.argsort(-logits, axis=1, kind="stable")
    top_idx = order[:, :TOP_K]                               # [N, 2]
    top_lg = np.take_along_axis(logits, top_idx, axis=1)
    mx = top_lg.max(axis=1, keepdims=True)
    ex = np.exp(top_lg - mx)
    gates2 = (ex / ex.sum(axis=1, keepdims=True)).astype(np.float32)
    return top_idx, gates2


def _run(inputs, trace=False):
    from concourse.bass_utils import run_bass_kernel_spmd

    bf16 = ml_dtypes.bfloat16
    h = np.asarray(inputs["h"], dtype=np.float32)
    w_gate = np.asarray(inputs["w_gate"], dtype=np.float32)
    w1 = np.asarray(inputs["w1"], dtype=np.float32)
    b1 = np.asarray(inputs["b1"], dtype=np.float32)
    w2 = np.asarray(inputs["w2"], dtype=np.float32)
    b2 = np.asarray(inputs["b2"], dtype=np.float32)
    N = h.shape[0]

    if b1.any() or b2.any():
        # safety fallback (setup_inputs uses zero biases): exact host compute
        logits = h @ w_gate
        order = np.argsort(-logits, axis=1, kind="stable")
        ti = order[:, :TOP_K]
        tl = np.take_along_axis(logits, ti, axis=1)
        exl = np.exp(tl - tl.max(axis=1, keepdims=True))
        g2 = exl / exl.sum(axis=1, keepdims=True)
        out = np.zeros((N, D), dtype=np.float32)
        for e in range(E):
            sel = ti == e
            toks = np.nonzero(sel.any(axis=1))[0]
            g = g2[toks, sel[toks].argmax(axis=1)]
            mid = np.maximum(h[toks] @ w1[e] + b1[e], 0.0)
            out[toks] += (mid @ w2[e] + b2[e]) * g[:, None]
        return out, None

    top_idx, gates2 = _route(h, w_gate)

    # dispatch: expert e -> cores 2e (first half) and 2e+1 (second half);
    # capacity-limited with host-side overflow handling
    core_toks, core_gates, core_expert, host_jobs = [], [], [], []
    for e in range(E):
        sel = top_idx == e                                   # [N, 2] bool
        toks = np.nonzero(sel.any(axis=1))[0]
        g = gates2[toks, sel[toks].argmax(axis=1)]
        if len(toks) > 2 * CAP:
            host_jobs.append((e, toks[2 * CAP:], g[2 * CAP:]))
            toks, g = toks[:2 * CAP], g[:2 * CAP]
        half = (len(toks) + 1) // 2
        for lo, hi in ((0, half), (half, len(toks))):
            core_toks.append(toks[lo:hi])
            core_gates.append(g[lo:hi])
            core_expert.append(e)

    maxlen = max(len(t) for t in core_toks)
    C = max(128, -(-maxlen // 2) * 2)

    if C not in _NC_CACHE:
        _NC_CACHE[C] = _build_moe_nc(C)
    nc = _NC_CACHE[C]

    chunks = _chunk_plan(C)
    s0 = chunks[0][1]

    # partition-major packers matching the kernel's flat DMA layouts
    def pack_w1(e, h0, h1):
        # di-major inside each h-tile: [128, (h1-h0)/128 * ND * 128]
        w1e = w1[e]
        if h1 > F8_HT0 * 128:
            w1e = w1e.copy()
            w1e[:, F8_HT0 * 128:] *= S_MID
        blk = (w1e.astype(bf16).reshape(ND, 128, NH, 128)
               [:, :, h0 // 128:h1 // 128, :])          # [ND,128,nht,128]
        return np.ascontiguousarray(
            blk.transpose(1, 2, 0, 3).reshape(128, -1))  # ht-major, di inside

    f8e4 = ml_dtypes.float8_e4m3fn
    w1_ht0 = {}
    w1_pieces = {}
    w2_packed = {}
    w2f8_packed = {}
    for e in set(core_expert):
        w1_ht0[e] = pack_w1(e, 0, N_HEAD_HT * 128)
        w1_pieces[e] = {
            (a, b): pack_w1(e, a * 128, (b + 1) * 128) for (a, b) in W1_PIECES}
        w2e = w2[e].copy()
        w2e[F8_HT0 * 128:] *= 1.0 / S_MID       # inverse of the w1 scaling
        wt = w2e.astype(bf16).reshape(16, 128, 512)   # [ht, k, d]
        w2_packed[e] = np.ascontiguousarray(
            wt[:14].transpose(1, 0, 2).reshape(128, 14 * 512))
        w2f8_packed[e] = np.ascontiguousarray(
            np.clip(w2e[F8_HT0 * 128:].reshape(N_F8_HT, 128, 512),
                    -240, 240).transpose(1, 0, 2).astype(f8e4))

    in_maps = []
    for c in range(N_CORES):
        e = core_expert[c]
        toks = core_toks[c]
        n = len(toks)
        xtT = np.zeros((D, C), dtype=bf16)
        xtT[:, :n] = h[toks].T.astype(bf16)
        r = xtT.reshape(ND, 128, C)

        def xt_block(c0, S):
            return r[:, :, c0:c0 + S].transpose(1, 0, 2).reshape(128, ND * S)

        hw = N_HEAD_HT * ND * 128
        xt0 = xt_block(*chunks[0]).reshape(128, ND, s0)
        head_a_arr = np.empty((128, hw + N_HEAD_DI * s0), dtype=bf16)
        head_a_arr[:, :hw] = w1_ht0[e]
        head_a_arr[:, hw:] = xt0[:, :N_HEAD_DI, :].reshape(128, -1)
        head_b_arr = np.ascontiguousarray(
            xt0[:, N_HEAD_DI:, :].reshape(128, -1))
        im = {"head_a": head_a_arr, "head_b": head_b_arr,
              "w2": w2_packed[e], "w2f8": w2f8_packed[e]}
        for (a, b) in W1_PIECES:
            im[f"w1p_{a}"] = w1_pieces[e][(a, b)]
        if C > s0:
            xt_arr = np.empty((128, ND * (C - s0)), dtype=bf16)
            o = 0
            for (c0, S) in chunks[1:]:
                xt_arr[:, o:o + ND * S] = xt_block(c0, S)
                o += ND * S
            im["xt"] = xt_arr
        in_maps.append(im)

    res = run_bass_kernel_spmd(nc, in_maps, core_ids=list(range(N_CORES)),
                               trace=trace)

    out = np.zeros((N, D), dtype=np.float32)
    # routing-overflow stragglers: same FFN on host, exact f32
    for e, toks, g in host_jobs:
        mid = np.maximum(h[toks] @ w1[e] + b1[e], 0.0)
        out[toks] += (mid @ w2[e] + b2[e]) * g[:, None]
    for c in range(N_CORES):
        toks = core_toks[c]
        if not len(toks):
            continue
        # unpack chunk-major [128, ND*C] back to y^T [D, C]
        raw = res.results[c]["yt"]
        ytT = np.empty((D, C), dtype=np.float32)
        o = 0
        for (c0, S) in chunks:
            ytT[:, c0:c0 + S] = (
                raw[:, o:o + ND * S].astype(np.float32).reshape(128, ND, S)
                .transpose(1, 0, 2).reshape(D, S))
            o += ND * S
        out[toks] += ytT[:, :len(toks)].T * core_gates[c][:, None]
    return out, res


def kernel(**inputs) -> np.ndarray:
    out, _ = _run(inputs, trace=False)
    return out
